# revision 8
# baseline (speedup 1.0000x reference)
"""Bidirectional Mamba block kernel for Trainium2 (8 NeuronCores).

Sharding: 8 cores = 4 batches x 2 directions. Each core runs the full Mamba
pass for one (batch, direction) pair; the backward direction's input is
flipped on the host. Zero inter-core communication.

Per-core pipeline:
  LN -> PE transpose -> in_proj (fp32r matmul) -> causal conv (gpsimd) ->
  silu -> x_proj/dt_proj -> softplus -> per (d-tile, n):
     dA = exp(A[d,n]*dt)  [ACT, per-partition scale]
     data1 = (dt*xc) . B_n [gpsimd, broadcast B]
     h = tensor_tensor_scan(dA, data1)  [DVE native scan]
     y += C_n . h  [DVE mult + PE identity-matmul accumulate into PSUM]
  gating (D*xs + y)*silu(z) -> out_proj (bf16 matmul) -> y^T to DRAM.
"""

import sys

sys.path.insert(0, "/opt/trn_rl_repo")

import numpy as np

D_MODEL = 1024
D_INNER = 2048
D_STATE = 16
D_CONV = 4
DT_RANK = 64
EPS = 1e-5
L = 2048
NB = 4
NCORES = 8
NDT = D_INNER // 128   # 16 d-tiles
NMT = D_MODEL // 128   # 8 dm-tiles
NTT = L // 128         # 16 t-tiles
TH = 2                 # t-halves for the scan phase
TC = L // TH           # 1024

_CACHE = {}


def _make_tc_class(tile, bass_rust, mybir):
    from concourse.vector_clock import ScopedClock

    class TC(tile.TileContext):
        """TileContext patched for this walrus build: max ONE sync wait per
        instruction (excess waits hoisted onto preceding same-engine NOPs,
        and the tail drain split into single-wait drains)."""

        def _add_instruction(self, inst):
            si = getattr(inst, "sync_info", None)
            if (
                si is not None
                and si.on_wait
                and len(si.on_wait) > 1
                and inst.engine != mybir.EngineType.Unassigned
            ):
                waits = list(si.on_wait)
                inst.sync_info = bass_rust.SyncInfo(
                    on_wait=[waits[-1]], on_update=list(si.on_update or [])
                )
                eng = self.nc.engines[inst.engine]
                for w in waits[:-1]:
                    nop = eng.nop(nofuse=True)
                    nop.ins.sync_info = bass_rust.SyncInfo(on_wait=[w], on_update=[])
            super()._add_instruction(inst)

        def _drain_and_barrier(self, tick_clock, wait_clock):
            nc = self.nc
            d = nc.sync.drain()
            wait_clock.add_sem_waits(
                d.ins, ScopedClock({None: tick_clock.global_clock})
            )
            si = d.ins.sync_info
            if si is not None and si.on_wait and len(si.on_wait) > 1:
                waits = list(si.on_wait)
                d.ins.sync_info = bass_rust.SyncInfo(
                    on_wait=waits[:1], on_update=list(si.on_update or [])
                )
                for i in range(1, len(waits)):
                    e = nc.sync.drain()
                    e.ins.sync_info = bass_rust.SyncInfo(
                        on_wait=waits[i : i + 1], on_update=[]
                    )
            nc.all_engine_barrier()
            assert self.sems is not None
            popped = nc._tile_sem_poison_stack.pop()
            assert popped is self._sem_poison
            nc.clear_and_free_semaphores(list(self.sems.allocated().values()))
            nc.all_engine_barrier()

    return TC


def _broadcast_ap(bass, handle_ap, parts=128):
    """Prepend a stride-0 partition dim to a DRAM access pattern."""
    return bass.AP(
        tensor=handle_ap.tensor,
        offset=handle_ap.offset,
        ap=[[0, parts]] + [list(x) for x in handle_ap.ap],
    )


def _build():
    import bass_rust
    import concourse.bass as bass
    import concourse.mybir as mybir
    import concourse.tile as tile
    from concourse.masks import make_identity

    TCC = _make_tc_class(tile, bass_rust, mybir)

    f32 = mybir.dt.float32
    f32r = mybir.dt.float32r
    bf = mybir.dt.bfloat16
    AO = mybir.AluOpType
    AF = mybir.ActivationFunctionType

    nc = bass.Bass()

    # ---- DRAM I/O ----
    xin = nc.declare_dram_parameter("xin", [L, D_MODEL], f32, isOutput=False)
    ln_g = nc.declare_dram_parameter("ln_g", [D_MODEL], f32, isOutput=False)
    ln_b = nc.declare_dram_parameter("ln_b", [D_MODEL], f32, isOutput=False)
    ipwT = nc.declare_dram_parameter("ipwT", [D_MODEL, 2 * D_INNER], f32r, isOutput=False)
    xpwT = nc.declare_dram_parameter("xpwT", [D_INNER, 96], f32r, isOutput=False)
    dtwT = nc.declare_dram_parameter("dtwT", [DT_RANK, D_INNER], f32, isOutput=False)
    dtb = nc.declare_dram_parameter("dtb", [D_INNER], f32, isOutput=False)
    convw = nc.declare_dram_parameter("convw", [D_INNER, D_CONV], f32, isOutput=False)
    convb = nc.declare_dram_parameter("convb", [D_INNER], f32, isOutput=False)
    alog = nc.declare_dram_parameter("alog", [D_INNER, D_STATE], f32, isOutput=False)
    dvec_d = nc.declare_dram_parameter("dvec", [D_INNER], f32, isOutput=False)
    owT = nc.declare_dram_parameter("owT", [D_INNER, D_MODEL], f32, isOutput=False)
    yT = nc.declare_dram_parameter("yT", [D_MODEL, L], f32, isOutput=True)

    # ---- DRAM scratch ----
    xs_sp = nc.dram_tensor("xs_sp", [NDT, 128, L], bf)
    zs_sp = nc.dram_tensor("zs_sp", [NDT, 128, L], bf)
    ow16 = nc.dram_tensor("ow16", [NDT, 128, D_MODEL], bf)
    dblbc = nc.dram_tensor("dblbc", [2 * D_STATE, L], bf)

    with TCC(nc) as tc:
        import contextlib

        est = contextlib.ExitStack()
        with est:
            consts = est.enter_context(tc.tile_pool(name="consts", bufs=1))

            # identities
            ident32 = consts.tile([128, 128], f32)
            make_identity(nc, ident32)
            ident16 = consts.tile([128, 128], bf)
            make_identity(nc, ident16)

            # A = -exp(A_log) as [128, NDT, D_STATE]
            a_raw = consts.tile([128, NDT, D_STATE], f32)
            nc.sync.dma_start(
                out=a_raw, in_=alog[:, :].rearrange("(o p) n -> p o n", p=128)
            )
            a_neg = consts.tile([128, NDT, D_STATE], f32)
            nc.scalar.activation(out=a_neg, in_=a_raw, func=AF.Exp)
            nc.scalar.mul(out=a_neg, in_=a_neg, mul=-1.0)

            cw = consts.tile([128, NDT, D_CONV], f32)
            nc.sync.dma_start(
                out=cw, in_=convw[:, :].rearrange("(o p) k -> p o k", p=128)
            )
            cb = consts.tile([128, NDT], f32)
            nc.sync.dma_start(out=cb, in_=convb[:].rearrange("(o p) -> p o", p=128))
            dtbt = consts.tile([128, NDT], f32)
            nc.sync.dma_start(out=dtbt, in_=dtb[:].rearrange("(o p) -> p o", p=128))
            dvec = consts.tile([128, NDT], f32)
            nc.sync.dma_start(out=dvec, in_=dvec_d[:].rearrange("(o p) -> p o", p=128))
            lng = consts.tile([128, NMT], f32)
            nc.sync.dma_start(out=lng, in_=ln_g[:].rearrange("(j p) -> p j", p=128))
            lnb = consts.tile([128, NMT], f32)
            nc.sync.dma_start(out=lnb, in_=ln_b[:].rearrange("(j p) -> p j", p=128))
            epst = consts.tile([128, 1], f32)
            nc.vector.memset(epst, EPS)
            onest = consts.tile([128, 1], f32)
            nc.vector.memset(onest, 1.0)

            dtw_sb = consts.tile([DT_RANK, D_INNER], f32)
            nc.sync.dma_start(out=dtw_sb, in_=dtwT[:, :])
            xpw = consts.tile([128, NDT, 96], f32r)
            nc.sync.dma_start(
                out=xpw, in_=xpwT[:, :].rearrange("(o p) r -> p o r", p=128)
            )

            # cross-phase persistent
            hstate = consts.tile([128, NDT, D_STATE], bf)
            dblT = consts.tile([96, L], f32)
            dblBC = consts.tile([2 * D_STATE, L], bf)

            # -------- Phase 0: out_proj weights -> bf16 bounce to DRAM --------
            with tc.tile_pool(name="pb_ow", bufs=2) as pb_ow:
                for o in range(NDT):
                    ow32 = pb_ow.tile([128, D_MODEL], f32, tag="ow32")
                    nc.sync.dma_start(
                        out=ow32, in_=owT[128 * o : 128 * (o + 1), :]
                    )
                    ow16t = pb_ow.tile([128, D_MODEL], bf, tag="ow16t")
                    nc.vector.tensor_copy(out=ow16t, in_=ow32)
                    nc.sync.dma_start(out=ow16[o], in_=ow16t)

            # ---------------- Phase A: LN + transpose ----------------
            with contextlib.ExitStack() as stAB:
                uTp = stAB.enter_context(tc.tile_pool(name="uT", bufs=1))
                uT = [
                    uTp.tile([128, L], f32r, tag=f"uT{j}", name=f"uT{j}")
                    for j in range(NMT)
                ]

                stA = stAB.enter_context(contextlib.ExitStack())
                pa_x = stA.enter_context(tc.tile_pool(name="pa_x", bufs=3))
                pa_u = stA.enter_context(tc.tile_pool(name="pa_u", bufs=2))
                pa_s = stA.enter_context(tc.tile_pool(name="pa_s", bufs=4))
                psA = stA.enter_context(
                    tc.tile_pool(name="psA", bufs=2, space="PSUM")
                )

                for i in range(NTT):
                    xt = pa_x.tile([128, D_MODEL], f32)
                    nc.sync.dma_start(out=xt, in_=xin[128 * i : 128 * (i + 1), :])
                    st = pa_s.tile([128, 2, 6], f32)
                    nc.vector.bn_stats(out=st[:, 0, :], in_=xt[:, 0:512])
                    nc.vector.bn_stats(out=st[:, 1, :], in_=xt[:, 512:1024])
                    mv = pa_s.tile([128, 2], f32)
                    nc.vector.bn_aggr(out=mv, in_=st)
                    rstd = pa_s.tile([128, 1], f32)
                    nc.scalar.activation(
                        out=rstd, in_=mv[:, 1:2], func=AF.Ln, bias=epst
                    )
                    nc.scalar.activation(
                        out=rstd, in_=rstd, func=AF.Exp, scale=-0.5
                    )
                    u = pa_u.tile([128, D_MODEL], f32)
                    nc.vector.tensor_scalar(
                        out=u,
                        in0=xt,
                        scalar1=mv[:, 0:1],
                        scalar2=rstd,
                        op0=AO.subtract,
                        op1=AO.mult,
                    )
                    for j in range(NMT):
                        pst = psA.tile([128, 128], f32)
                        nc.tensor.transpose(
                            pst, u[:, 128 * j : 128 * (j + 1)], ident32
                        )
                        nc.scalar.activation(
                            out=uT[j][:, 128 * i : 128 * (i + 1)],
                            in_=pst,
                            func=AF.Identity,
                            bias=lnb[:, j : j + 1],
                            scale=lng[:, j : j + 1],
                        )

                stA.close()

                # ------------- Phase B: in_proj/conv/silu/x_proj -------------
                with contextlib.ExitStack() as stB:
                    ipw = stB.enter_context(tc.tile_pool(name="ipw", bufs=2))
                    pb_xcT = stB.enter_context(tc.tile_pool(name="pb_xcT", bufs=2))
                    pb_xcv = stB.enter_context(tc.tile_pool(name="pb_xcv", bufs=2))
                    pb_xs32 = stB.enter_context(tc.tile_pool(name="pb_xs32", bufs=2))
                    pb_x16 = stB.enter_context(tc.tile_pool(name="pb_x16", bufs=3))
                    psB = stB.enter_context(
                        tc.tile_pool(name="psB", bufs=2, space="PSUM")
                    )
                    psD = stB.enter_context(
                        tc.tile_pool(name="psD", bufs=1, space="PSUM")
                    )

                    dbl_ps = psD.tile([96, L], f32)

                    for o in range(2 * NDT):
                        is_x = o < NDT
                        wts = []
                        for j in range(NMT):
                            wt = ipw.tile([128, 128], f32r, tag=f"w{j}")
                            nc.sync.dma_start(
                                out=wt,
                                in_=ipwT[
                                    128 * j : 128 * (j + 1),
                                    128 * o : 128 * (o + 1),
                                ],
                            )
                            wts.append(wt)
                        if is_x:
                            xcT = pb_xcT.tile([128, L + D_CONV - 1], f32)
                            nc.vector.memset(xcT[:, 0 : D_CONV - 1], 0.0)
                        else:
                            zs16 = pb_x16.tile([128, L], bf, tag="zs16")
                        for half in range(2):
                            psb = psB.tile([128, TC], f32)
                            for nn2 in range(2):
                                ncol = 512 * nn2
                                for j in range(NMT):
                                    nc.tensor.matmul(
                                        psb[:, ncol : ncol + 512],
                                        lhsT=wts[j],
                                        rhs=uT[j][
                                            :,
                                            TC * half + ncol : TC * half + ncol + 512,
                                        ],
                                        start=(j == 0),
                                        stop=(j == NMT - 1),
                                    )
                            if is_x:
                                nc.scalar.copy(
                                    out=xcT[
                                        :,
                                        D_CONV - 1 + TC * half : D_CONV - 1 + TC * (half + 1),
                                    ],
                                    in_=psb,
                                )
                            else:
                                zt = pb_xs32.tile([128, TC], f32, tag="sg", name="zt")
                                nc.scalar.activation(
                                    out=zt, in_=psb, func=AF.Exp, scale=-1.0
                                )
                                nc.scalar.activation(
                                    out=zt, in_=zt, func=AF.Ln, bias=onest
                                )
                                nc.scalar.activation(
                                    out=zt, in_=zt, func=AF.Exp, scale=-1.0
                                )
                                nc.vector.tensor_mul(
                                    out=zs16[:, TC * half : TC * (half + 1)],
                                    in0=psb,
                                    in1=zt,
                                )
                        if is_x:
                            # depthwise causal conv along t (gpsimd)
                            xcv = pb_xcv.tile([128, L], f32)
                            nc.vector.tensor_scalar(
                                out=xcv,
                                in0=xcT[:, 0:L],
                                scalar1=cw[:, o, 0:1],
                                scalar2=cb[:, o : o + 1],
                                op0=AO.mult,
                                op1=AO.add,
                            )
                            for k in range(1, D_CONV):
                                nc.vector.scalar_tensor_tensor(
                                    out=xcv,
                                    in0=xcT[:, k : k + L],
                                    scalar=cw[:, o, k : k + 1],
                                    in1=xcv,
                                    op0=AO.mult,
                                    op1=AO.add,
                                )
                            sg = pb_xs32.tile([128, L], f32, tag="sg")
                            nc.scalar.activation(
                                out=sg, in_=xcv, func=AF.Exp, scale=-1.0
                            )
                            nc.scalar.activation(
                                out=sg, in_=sg, func=AF.Ln, bias=onest
                            )
                            nc.scalar.activation(
                                out=sg, in_=sg, func=AF.Exp, scale=-1.0
                            )
                            xs32 = pb_xs32.tile([128, L], f32r)
                            nc.vector.tensor_mul(out=xs32, in0=xcv, in1=sg)
                            xs16 = pb_x16.tile([128, L], bf, tag="xs16")
                            nc.vector.tensor_copy(
                                out=xs16, in_=xs32.bitcast(f32)
                            )
                            nc.sync.dma_start(out=xs_sp[o], in_=xs16)
                            for nn in range(4):
                                nc.tensor.matmul(
                                    dbl_ps[:, 512 * nn : 512 * (nn + 1)],
                                    lhsT=xpw[:, o, :],
                                    rhs=xs32[:, 512 * nn : 512 * (nn + 1)],
                                    start=(o == 0),
                                    stop=(o == NDT - 1),
                                    skip_group_check=True,
                                )
                        else:
                            nc.sync.dma_start(out=zs_sp[o - NDT], in_=zs16)

                    # evacuate dbl
                    nc.scalar.copy(out=dblT, in_=dbl_ps)
                    nc.vector.tensor_copy(out=dblBC, in_=dblT[DT_RANK:96, :])
                    nc.sync.dma_start(out=dblbc[:, :], in_=dblBC)

            # ---------------- Phase C+D: scan + out_proj ----------------
            with contextlib.ExitStack() as stC:
                pc_bc = stC.enter_context(tc.tile_pool(name="pc_bc", bufs=1))
                pc_dt = stC.enter_context(tc.tile_pool(name="pc_dt", bufs=2))
                pc_io = stC.enter_context(tc.tile_pool(name="pc_io", bufs=3))
                pc_w = stC.enter_context(tc.tile_pool(name="pc_w", bufs=2))
                pc_da = stC.enter_context(tc.tile_pool(name="pc_da", bufs=3))
                pc_d1 = stC.enter_context(tc.tile_pool(name="pc_d1", bufs=3))
                pc_h = stC.enter_context(tc.tile_pool(name="pc_h", bufs=3))
                pc_tmp = stC.enter_context(tc.tile_pool(name="pc_tmp", bufs=3))
                pc_g = stC.enter_context(tc.tile_pool(name="pc_g", bufs=2))
                pc_yg = stC.enter_context(tc.tile_pool(name="pc_yg", bufs=1))
                pd_w = stC.enter_context(tc.tile_pool(name="pd_w", bufs=3))
                ps_dt = stC.enter_context(
                    tc.tile_pool(name="ps_dt", bufs=1, space="PSUM")
                )
                ps_y = stC.enter_context(
                    tc.tile_pool(name="ps_y", bufs=2, space="PSUM")
                )
                ps_o = stC.enter_context(
                    tc.tile_pool(name="ps_o", bufs=2, space="PSUM")
                )

                for th in range(TH):
                    tsl = slice(TC * th, TC * (th + 1))
                    B_bc = pc_bc.tile([128, D_STATE, TC], bf, tag="Bbc")
                    nc.sync.dma_start(
                        out=B_bc,
                        in_=_broadcast_ap(bass, dblbc[0:D_STATE, tsl]),
                    )
                    C_bc = pc_bc.tile([128, D_STATE, TC], bf, tag="Cbc")
                    nc.sync.dma_start(
                        out=C_bc,
                        in_=_broadcast_ap(bass, dblbc[D_STATE : 2 * D_STATE, tsl]),
                    )
                    yg_all = pc_yg.tile([128, NDT, TC], bf, tag="yg")

                    for o in range(NDT):
                        dt_ps = ps_dt.tile([128, TC], f32)
                        for nn2 in range(2):
                            nc.tensor.matmul(
                                dt_ps[:, 512 * nn2 : 512 * (nn2 + 1)],
                                lhsT=dtw_sb[:, 128 * o : 128 * (o + 1)],
                                rhs=dblT[
                                    0:DT_RANK,
                                    TC * th + 512 * nn2 : TC * th + 512 * (nn2 + 1),
                                ],
                                start=True,
                                stop=True,
                            )
                        dt_o = pc_dt.tile([128, TC], f32)
                        nc.scalar.activation(
                            out=dt_o,
                            in_=dt_ps,
                            func=AF.Exp,
                            bias=dtbt[:, o : o + 1],
                        )
                        nc.scalar.activation(
                            out=dt_o, in_=dt_o, func=AF.Ln, bias=onest
                        )
                        xs_o = pc_io.tile([128, TC], bf, tag="xs_o")
                        nc.sync.dma_start(out=xs_o, in_=xs_sp[o, :, tsl])
                        zs_o = pc_io.tile([128, TC], bf, tag="zs_o")
                        nc.sync.dma_start(out=zs_o, in_=zs_sp[o, :, tsl])
                        w_o = pc_w.tile([128, TC], bf)
                        nc.vector.tensor_mul(out=w_o, in0=dt_o, in1=xs_o)

                        y_ps = ps_y.tile([128, TC], f32)
                        for n in range(D_STATE):
                            dA = pc_da.tile([128, TC], f32)
                            nc.scalar.activation(
                                out=dA,
                                in_=dt_o,
                                func=AF.Exp,
                                scale=a_neg[:, o, n : n + 1],
                            )
                            d1 = pc_d1.tile([128, TC], bf)
                            nc.gpsimd.tensor_mul(
                                out=d1, in0=w_o, in1=B_bc[:, n, :]
                            )
                            h = pc_h.tile([128, TC], bf)
                            nc.vector.tensor_tensor_scan(
                                out=h,
                                data0=dA,
                                data1=d1,
                                initial=(
                                    0.0 if th == 0 else hstate[:, o, n : n + 1]
                                ),
                                op0=AO.mult,
                                op1=AO.add,
                            )
                            if th == 0:
                                nc.gpsimd.tensor_copy(
                                    out=hstate[:, o, n : n + 1],
                                    in_=h[:, TC - 1 : TC],
                                )
                            tmp = pc_tmp.tile([128, TC], bf)
                            nc.vector.tensor_mul(out=tmp, in0=h, in1=C_bc[:, n, :])
                            for nn2 in range(2):
                                nc.tensor.matmul(
                                    y_ps[:, 512 * nn2 : 512 * (nn2 + 1)],
                                    lhsT=ident16,
                                    rhs=tmp[:, 512 * nn2 : 512 * (nn2 + 1)],
                                    start=(n == 0),
                                    stop=(n == D_STATE - 1),
                                )
                        g1 = pc_g.tile([128, TC], bf)
                        nc.vector.scalar_tensor_tensor(
                            out=g1,
                            in0=xs_o,
                            scalar=dvec[:, o : o + 1],
                            in1=y_ps,
                            op0=AO.mult,
                            op1=AO.add,
                        )
                        nc.vector.tensor_mul(
                            out=yg_all[:, o, :], in0=g1, in1=zs_o
                        )

                    # ---- Phase D (out_proj) for this t-half ----
                    for m in range(NMT):
                        for nn2 in range(2):
                            op_ps = ps_o.tile([128, 512], f32)
                            for o in range(NDT):
                                owt = pd_w.tile([128, 128], bf)
                                nc.sync.dma_start(
                                    out=owt,
                                    in_=ow16[o, :, 128 * m : 128 * (m + 1)],
                                )
                                nc.tensor.matmul(
                                    op_ps,
                                    lhsT=owt,
                                    rhs=yg_all[:, o, 512 * nn2 : 512 * (nn2 + 1)],
                                    start=(o == 0),
                                    stop=(o == NDT - 1),
                                )
                            ot_sb = pd_w.tile([128, 512], f32, tag="ot_sb")
                            nc.scalar.copy(out=ot_sb, in_=op_ps)
                            nc.sync.dma_start(
                                out=yT[
                                    128 * m : 128 * (m + 1),
                                    TC * th + 512 * nn2 : TC * th + 512 * (nn2 + 1),
                                ],
                                in_=ot_sb,
                            )

    return nc


def _get_nc():
    if "nc" not in _CACHE:
        _CACHE["nc"] = _build()
    return _CACHE["nc"]


def kernel(x, ln1_g, ln1_b, ln2_g, ln2_b, in_proj_w, conv_w, conv_b,
           x_proj_w, dt_proj_w, dt_proj_b, A_log, D, out_proj_w):
    from concourse.bass_utils import run_bass_kernel_spmd

    x = np.asarray(x, np.float32)
    f = np.float32
    shared = {
        "ipwT": np.ascontiguousarray(np.asarray(in_proj_w, f).T),
        "xpwT": np.ascontiguousarray(np.asarray(x_proj_w, f).T),
        "dtwT": np.ascontiguousarray(np.asarray(dt_proj_w, f).T),
        "dtb": np.ascontiguousarray(np.asarray(dt_proj_b, f)),
        "convw": np.ascontiguousarray(np.asarray(conv_w, f)),
        "convb": np.ascontiguousarray(np.asarray(conv_b, f)),
        "alog": np.ascontiguousarray(np.asarray(A_log, f)),
        "dvec": np.ascontiguousarray(np.asarray(D, f)),
        "owT": np.ascontiguousarray(np.asarray(out_proj_w, f).T),
    }
    g1, b1 = np.asarray(ln1_g, f), np.asarray(ln1_b, f)
    g2, b2 = np.asarray(ln2_g, f), np.asarray(ln2_b, f)

    in_maps = []
    for c in range(NCORES):
        b = c % NB
        if c < NB:
            xi = np.ascontiguousarray(x[b])
            lg, lb = g1, b1
        else:
            xi = np.ascontiguousarray(x[b][::-1])
            lg, lb = g2, b2
        m = dict(shared)
        m["xin"] = xi
        m["ln_g"] = np.ascontiguousarray(lg)
        m["ln_b"] = np.ascontiguousarray(lb)
        in_maps.append(m)

    nc = _get_nc()
    res = run_bass_kernel_spmd(nc, in_maps, core_ids=list(range(NCORES)))
    _CACHE["last_results"] = res

    out = x.copy()
    for c in range(NCORES):
        b = c % NB
        yt = res.results[c]["yT"]  # [D_MODEL, L]
        if c < NB:
            out[b] += yt.T
        else:
            out[b] += yt.T[::-1]
    return out


if __name__ == "__main__":
    ins = dict(np.load("/root/problem/inputs.npz"))
    o = kernel(**ins)
    print("kernel ran, out shape", o.shape)


# revision 10
# speedup vs baseline: 1.3123x; 1.3123x over previous
"""Bidirectional Mamba block kernel for Trainium2 (8 NeuronCores).

Sharding: 8 cores = 4 batches x 2 directions. Each core runs the full Mamba
pass for one (batch, direction) pair; the backward direction's input is
flipped on the host. Zero inter-core communication.

Per-core pipeline:
  LN -> PE transpose -> in_proj (fp32r matmul) -> causal conv (gpsimd) ->
  silu -> x_proj/dt_proj -> softplus -> per (d-tile, n):
     dA = exp(A[d,n]*dt)  [ACT, per-partition scale]
     data1 = (dt*xc) . B_n [gpsimd, broadcast B]
     h = tensor_tensor_scan(dA, data1)  [DVE native scan]
     y += C_n . h  [DVE mult + PE identity-matmul accumulate into PSUM]
  gating (D*xs + y)*silu(z) -> out_proj (bf16 matmul) -> y^T to DRAM.
"""

import sys

sys.path.insert(0, "/opt/trn_rl_repo")

import numpy as np

D_MODEL = 1024
D_INNER = 2048
D_STATE = 16
D_CONV = 4
DT_RANK = 64
EPS = 1e-5
L = 2048
NB = 4
NCORES = 8
NDT = D_INNER // 128   # 16 d-tiles
NMT = D_MODEL // 128   # 8 dm-tiles
NTT = L // 128         # 16 t-tiles
TH = 2                 # t-halves for the scan phase
TC = L // TH           # 1024

_CACHE = {}


def _make_tc_class(tile, bass_rust, mybir):
    from concourse.vector_clock import ScopedClock

    class TC(tile.TileContext):
        """TileContext patched for this walrus build: max ONE sync wait per
        instruction (excess waits hoisted onto preceding same-engine NOPs,
        and the tail drain split into single-wait drains)."""

        def _add_instruction(self, inst):
            si = getattr(inst, "sync_info", None)
            if (
                si is not None
                and si.on_wait
                and len(si.on_wait) > 1
                and inst.engine != mybir.EngineType.Unassigned
            ):
                waits = list(si.on_wait)
                inst.sync_info = bass_rust.SyncInfo(
                    on_wait=[waits[-1]], on_update=list(si.on_update or [])
                )
                eng = self.nc.engines[inst.engine]
                for w in waits[:-1]:
                    nop = eng.nop(nofuse=True)
                    nop.ins.sync_info = bass_rust.SyncInfo(on_wait=[w], on_update=[])
            super()._add_instruction(inst)

        def _drain_and_barrier(self, tick_clock, wait_clock):
            nc = self.nc
            d = nc.sync.drain()
            wait_clock.add_sem_waits(
                d.ins, ScopedClock({None: tick_clock.global_clock})
            )
            si = d.ins.sync_info
            if si is not None and si.on_wait and len(si.on_wait) > 1:
                waits = list(si.on_wait)
                d.ins.sync_info = bass_rust.SyncInfo(
                    on_wait=waits[:1], on_update=list(si.on_update or [])
                )
                for i in range(1, len(waits)):
                    e = nc.sync.drain()
                    e.ins.sync_info = bass_rust.SyncInfo(
                        on_wait=waits[i : i + 1], on_update=[]
                    )
            nc.all_engine_barrier()
            assert self.sems is not None
            popped = nc._tile_sem_poison_stack.pop()
            assert popped is self._sem_poison
            nc.clear_and_free_semaphores(list(self.sems.allocated().values()))
            nc.all_engine_barrier()

    return TC


def _broadcast_ap(bass, handle_ap, parts=128):
    """Prepend a stride-0 partition dim to a DRAM access pattern."""
    return bass.AP(
        tensor=handle_ap.tensor,
        offset=handle_ap.offset,
        ap=[[0, parts]] + [list(x) for x in handle_ap.ap],
    )


def _build():
    import bass_rust
    import concourse.bass as bass
    import concourse.mybir as mybir
    import concourse.tile as tile
    from concourse.masks import make_identity

    TCC = _make_tc_class(tile, bass_rust, mybir)

    f32 = mybir.dt.float32
    f32r = mybir.dt.float32r
    bf = mybir.dt.bfloat16
    AO = mybir.AluOpType
    AF = mybir.ActivationFunctionType

    nc = bass.Bass()

    # ---- DRAM I/O ----
    xin = nc.declare_dram_parameter("xin", [L, D_MODEL], f32, isOutput=False)
    ln_g = nc.declare_dram_parameter("ln_g", [D_MODEL], f32, isOutput=False)
    ln_b = nc.declare_dram_parameter("ln_b", [D_MODEL], f32, isOutput=False)
    ipwT = nc.declare_dram_parameter("ipwT", [D_MODEL, 2 * D_INNER], f32r, isOutput=False)
    xpwT = nc.declare_dram_parameter("xpwT", [D_INNER, 96], f32r, isOutput=False)
    dtwT = nc.declare_dram_parameter("dtwT", [DT_RANK, D_INNER], f32, isOutput=False)
    dtb = nc.declare_dram_parameter("dtb", [D_INNER], f32, isOutput=False)
    convw = nc.declare_dram_parameter("convw", [D_INNER, D_CONV], f32, isOutput=False)
    convb = nc.declare_dram_parameter("convb", [D_INNER], f32, isOutput=False)
    alog = nc.declare_dram_parameter("alog", [D_INNER, D_STATE], f32, isOutput=False)
    dvec_d = nc.declare_dram_parameter("dvec", [D_INNER], f32, isOutput=False)
    owT = nc.declare_dram_parameter("owT", [D_INNER, D_MODEL], f32, isOutput=False)
    yT = nc.declare_dram_parameter("yT", [D_MODEL, L], f32, isOutput=True)

    # ---- DRAM scratch ----
    xs_sp = nc.dram_tensor("xs_sp", [NDT, 128, L], bf)
    zs_sp = nc.dram_tensor("zs_sp", [NDT, 128, L], bf)
    ow16 = nc.dram_tensor("ow16", [NDT, 128, D_MODEL], bf)
    dblbc = nc.dram_tensor("dblbc", [2 * D_STATE, L], bf)

    with TCC(nc) as tc:
        import contextlib

        est = contextlib.ExitStack()
        with est:
            consts = est.enter_context(tc.tile_pool(name="consts", bufs=1))

            # identities
            ident32 = consts.tile([128, 128], f32)
            make_identity(nc, ident32)
            ident16 = consts.tile([128, 128], bf)
            make_identity(nc, ident16)

            # A = -exp(A_log) as [128, NDT, D_STATE]
            a_raw = consts.tile([128, NDT, D_STATE], f32)
            nc.sync.dma_start(
                out=a_raw, in_=alog[:, :].rearrange("(o p) n -> p o n", p=128)
            )
            a_neg = consts.tile([128, NDT, D_STATE], f32)
            nc.scalar.activation(out=a_neg, in_=a_raw, func=AF.Exp)
            nc.scalar.mul(out=a_neg, in_=a_neg, mul=-1.0)

            cw = consts.tile([128, NDT, D_CONV], f32)
            nc.sync.dma_start(
                out=cw, in_=convw[:, :].rearrange("(o p) k -> p o k", p=128)
            )
            cb = consts.tile([128, NDT], f32)
            nc.sync.dma_start(out=cb, in_=convb[:].rearrange("(o p) -> p o", p=128))
            dtbt = consts.tile([128, NDT], f32)
            nc.sync.dma_start(out=dtbt, in_=dtb[:].rearrange("(o p) -> p o", p=128))
            dvec = consts.tile([128, NDT], f32)
            nc.sync.dma_start(out=dvec, in_=dvec_d[:].rearrange("(o p) -> p o", p=128))
            lng = consts.tile([128, NMT], f32)
            nc.sync.dma_start(out=lng, in_=ln_g[:].rearrange("(j p) -> p j", p=128))
            lnb = consts.tile([128, NMT], f32)
            nc.sync.dma_start(out=lnb, in_=ln_b[:].rearrange("(j p) -> p j", p=128))
            epst = consts.tile([128, 1], f32)
            nc.vector.memset(epst, EPS)
            onest = consts.tile([128, 1], f32)
            nc.vector.memset(onest, 1.0)

            dtw_sb = consts.tile([DT_RANK, D_INNER], f32)
            nc.sync.dma_start(out=dtw_sb, in_=dtwT[:, :])
            xpw = consts.tile([128, NDT, 96], f32r)
            nc.sync.dma_start(
                out=xpw, in_=xpwT[:, :].rearrange("(o p) r -> p o r", p=128)
            )

            # cross-phase persistent
            hstate = consts.tile([128, NDT, D_STATE], bf)
            dblT = consts.tile([96, L], f32)
            dblBC = consts.tile([2 * D_STATE, L], bf)

            # -------- Phase 0: out_proj weights -> bf16 bounce to DRAM --------
            with tc.tile_pool(name="pb_ow", bufs=2) as pb_ow:
                for o in range(NDT):
                    ow32 = pb_ow.tile([128, D_MODEL], f32, tag="ow32")
                    nc.sync.dma_start(
                        out=ow32, in_=owT[128 * o : 128 * (o + 1), :]
                    )
                    ow16t = pb_ow.tile([128, D_MODEL], bf, tag="ow16t")
                    nc.vector.tensor_copy(out=ow16t, in_=ow32)
                    nc.sync.dma_start(out=ow16[o], in_=ow16t)

            # ---------------- Phase A: LN + transpose ----------------
            with contextlib.ExitStack() as stAB:
                uTp = stAB.enter_context(tc.tile_pool(name="uT", bufs=1))
                uT = [
                    uTp.tile([128, L], f32r, tag=f"uT{j}", name=f"uT{j}")
                    for j in range(NMT)
                ]

                stA = stAB.enter_context(contextlib.ExitStack())
                pa_x = stA.enter_context(tc.tile_pool(name="pa_x", bufs=3))
                pa_u = stA.enter_context(tc.tile_pool(name="pa_u", bufs=2))
                pa_s = stA.enter_context(tc.tile_pool(name="pa_s", bufs=4))
                psA = stA.enter_context(
                    tc.tile_pool(name="psA", bufs=2, space="PSUM")
                )

                for i in range(NTT):
                    xt = pa_x.tile([128, D_MODEL], f32)
                    nc.sync.dma_start(out=xt, in_=xin[128 * i : 128 * (i + 1), :])
                    st = pa_s.tile([128, 2, 6], f32)
                    nc.vector.bn_stats(out=st[:, 0, :], in_=xt[:, 0:512])
                    nc.vector.bn_stats(out=st[:, 1, :], in_=xt[:, 512:1024])
                    mv = pa_s.tile([128, 2], f32)
                    nc.vector.bn_aggr(out=mv, in_=st)
                    rstd = pa_s.tile([128, 1], f32)
                    nc.scalar.activation(
                        out=rstd, in_=mv[:, 1:2], func=AF.Ln, bias=epst
                    )
                    nc.scalar.activation(
                        out=rstd, in_=rstd, func=AF.Exp, scale=-0.5
                    )
                    u = pa_u.tile([128, D_MODEL], f32)
                    nc.vector.tensor_scalar(
                        out=u,
                        in0=xt,
                        scalar1=mv[:, 0:1],
                        scalar2=rstd,
                        op0=AO.subtract,
                        op1=AO.mult,
                    )
                    for j in range(NMT):
                        pst = psA.tile([128, 128], f32)
                        nc.tensor.transpose(
                            pst, u[:, 128 * j : 128 * (j + 1)], ident32
                        )
                        nc.scalar.activation(
                            out=uT[j][:, 128 * i : 128 * (i + 1)],
                            in_=pst,
                            func=AF.Identity,
                            bias=lnb[:, j : j + 1],
                            scale=lng[:, j : j + 1],
                        )

                stA.close()

                # ------------- Phase B: in_proj/conv/silu/x_proj -------------
                with contextlib.ExitStack() as stB:
                    ipw = stB.enter_context(tc.tile_pool(name="ipw", bufs=2))
                    pb_xcT = stB.enter_context(tc.tile_pool(name="pb_xcT", bufs=2))
                    pb_xcv = stB.enter_context(tc.tile_pool(name="pb_xcv", bufs=2))
                    pb_xs32 = stB.enter_context(tc.tile_pool(name="pb_xs32", bufs=2))
                    pb_x16 = stB.enter_context(tc.tile_pool(name="pb_x16", bufs=3))
                    psB = stB.enter_context(
                        tc.tile_pool(name="psB", bufs=2, space="PSUM")
                    )
                    psD = stB.enter_context(
                        tc.tile_pool(name="psD", bufs=1, space="PSUM")
                    )

                    dbl_ps = psD.tile([96, L], f32)

                    for o in range(2 * NDT):
                        is_x = o < NDT
                        wt8 = ipw.tile([128, NMT, 128], f32r, tag="wt8")
                        nc.sync.dma_start(
                            out=wt8,
                            in_=ipwT[:, 128 * o : 128 * (o + 1)].rearrange(
                                "(j p) c -> p j c", p=128
                            ),
                        )
                        wts = [wt8[:, j, :] for j in range(NMT)]
                        if is_x:
                            xcT = pb_xcT.tile([128, L + D_CONV - 1], f32)
                            nc.vector.memset(xcT[:, 0 : D_CONV - 1], 0.0)
                        else:
                            zs16 = pb_x16.tile([128, L], bf, tag="zs16")
                        for half in range(2):
                            psb = psB.tile([128, TC], f32)
                            for nn2 in range(2):
                                ncol = 512 * nn2
                                for j in range(NMT):
                                    nc.tensor.matmul(
                                        psb[:, ncol : ncol + 512],
                                        lhsT=wts[j],
                                        rhs=uT[j][
                                            :,
                                            TC * half + ncol : TC * half + ncol + 512,
                                        ],
                                        start=(j == 0),
                                        stop=(j == NMT - 1),
                                    )
                            if is_x:
                                nc.scalar.copy(
                                    out=xcT[
                                        :,
                                        D_CONV - 1 + TC * half : D_CONV - 1 + TC * (half + 1),
                                    ],
                                    in_=psb,
                                )
                            else:
                                zt = pb_xs32.tile([128, TC], f32, tag="sg", name="zt")
                                nc.scalar.activation(
                                    out=zt, in_=psb, func=AF.Exp, scale=-1.0
                                )
                                nc.scalar.activation(
                                    out=zt, in_=zt, func=AF.Ln, bias=onest
                                )
                                nc.scalar.activation(
                                    out=zt, in_=zt, func=AF.Exp, scale=-1.0
                                )
                                nc.vector.tensor_mul(
                                    out=zs16[:, TC * half : TC * (half + 1)],
                                    in0=psb,
                                    in1=zt,
                                )
                        if is_x:
                            # depthwise causal conv along t (gpsimd)
                            xcv = pb_xcv.tile([128, L], f32)
                            nc.vector.tensor_scalar(
                                out=xcv,
                                in0=xcT[:, 0:L],
                                scalar1=cw[:, o, 0:1],
                                scalar2=cb[:, o : o + 1],
                                op0=AO.mult,
                                op1=AO.add,
                            )
                            for k in range(1, D_CONV):
                                nc.vector.scalar_tensor_tensor(
                                    out=xcv,
                                    in0=xcT[:, k : k + L],
                                    scalar=cw[:, o, k : k + 1],
                                    in1=xcv,
                                    op0=AO.mult,
                                    op1=AO.add,
                                )
                            sg = pb_xs32.tile([128, L], f32, tag="sg")
                            nc.scalar.activation(
                                out=sg, in_=xcv, func=AF.Exp, scale=-1.0
                            )
                            nc.scalar.activation(
                                out=sg, in_=sg, func=AF.Ln, bias=onest
                            )
                            nc.scalar.activation(
                                out=sg, in_=sg, func=AF.Exp, scale=-1.0
                            )
                            xs32 = pb_xs32.tile([128, L], f32r)
                            nc.vector.tensor_mul(out=xs32, in0=xcv, in1=sg)
                            xs16 = pb_x16.tile([128, L], bf, tag="xs16")
                            nc.vector.tensor_copy(
                                out=xs16, in_=xs32.bitcast(f32)
                            )
                            nc.sync.dma_start(out=xs_sp[o], in_=xs16)
                            for nn in range(4):
                                nc.tensor.matmul(
                                    dbl_ps[:, 512 * nn : 512 * (nn + 1)],
                                    lhsT=xpw[:, o, :],
                                    rhs=xs32[:, 512 * nn : 512 * (nn + 1)],
                                    start=(o == 0),
                                    stop=(o == NDT - 1),
                                    skip_group_check=True,
                                )
                        else:
                            nc.sync.dma_start(out=zs_sp[o - NDT], in_=zs16)

                    # evacuate dbl
                    nc.scalar.copy(out=dblT, in_=dbl_ps)
                    nc.vector.tensor_copy(out=dblBC, in_=dblT[DT_RANK:96, :])
                    nc.sync.dma_start(out=dblbc[:, :], in_=dblBC)

            # ---------------- Phase C+D: scan + out_proj ----------------
            with contextlib.ExitStack() as stC:
                pc_bc = stC.enter_context(tc.tile_pool(name="pc_bc", bufs=1))
                pc_dt = stC.enter_context(tc.tile_pool(name="pc_dt", bufs=2))
                pc_io = stC.enter_context(tc.tile_pool(name="pc_io", bufs=3))
                pc_w = stC.enter_context(tc.tile_pool(name="pc_w", bufs=2))
                pc_da = stC.enter_context(tc.tile_pool(name="pc_da", bufs=3))
                pc_d1 = stC.enter_context(tc.tile_pool(name="pc_d1", bufs=3))
                pc_h = stC.enter_context(tc.tile_pool(name="pc_h", bufs=3))
                pc_tmp = stC.enter_context(tc.tile_pool(name="pc_tmp", bufs=3))
                pc_g = stC.enter_context(tc.tile_pool(name="pc_g", bufs=2))
                pc_yg = stC.enter_context(tc.tile_pool(name="pc_yg", bufs=1))
                pd_w = stC.enter_context(tc.tile_pool(name="pd_w", bufs=3))
                ps_dt = stC.enter_context(
                    tc.tile_pool(name="ps_dt", bufs=1, space="PSUM")
                )
                ps_y = stC.enter_context(
                    tc.tile_pool(name="ps_y", bufs=2, space="PSUM")
                )
                ps_o = stC.enter_context(
                    tc.tile_pool(name="ps_o", bufs=1, space="PSUM")
                )

                for th in range(TH):
                    tsl = slice(TC * th, TC * (th + 1))
                    B_bc = pc_bc.tile([128, D_STATE, TC], bf, tag="Bbc")
                    nc.sync.dma_start(
                        out=B_bc,
                        in_=_broadcast_ap(bass, dblbc[0:D_STATE, tsl]),
                    )
                    C_bc = pc_bc.tile([128, D_STATE, TC], bf, tag="Cbc")
                    nc.sync.dma_start(
                        out=C_bc,
                        in_=_broadcast_ap(bass, dblbc[D_STATE : 2 * D_STATE, tsl]),
                    )
                    yg_all = pc_yg.tile([128, NDT, TC], bf, tag="yg")

                    for o in range(NDT):
                        dt_ps = ps_dt.tile([128, TC], f32)
                        for nn2 in range(2):
                            nc.tensor.matmul(
                                dt_ps[:, 512 * nn2 : 512 * (nn2 + 1)],
                                lhsT=dtw_sb[:, 128 * o : 128 * (o + 1)],
                                rhs=dblT[
                                    0:DT_RANK,
                                    TC * th + 512 * nn2 : TC * th + 512 * (nn2 + 1),
                                ],
                                start=True,
                                stop=True,
                            )
                        dt_o = pc_dt.tile([128, TC], f32)
                        nc.scalar.activation(
                            out=dt_o,
                            in_=dt_ps,
                            func=AF.Exp,
                            bias=dtbt[:, o : o + 1],
                        )
                        nc.scalar.activation(
                            out=dt_o, in_=dt_o, func=AF.Ln, bias=onest
                        )
                        xs_o = pc_io.tile([128, TC], bf, tag="xs_o")
                        nc.sync.dma_start(out=xs_o, in_=xs_sp[o, :, tsl])
                        zs_o = pc_io.tile([128, TC], bf, tag="zs_o")
                        nc.sync.dma_start(out=zs_o, in_=zs_sp[o, :, tsl])
                        w_o = pc_w.tile([128, TC], bf)
                        nc.vector.tensor_mul(out=w_o, in0=dt_o, in1=xs_o)

                        y_ps = ps_y.tile([128, TC], f32)
                        for n in range(D_STATE):
                            dA = pc_da.tile([128, TC], f32)
                            nc.scalar.activation(
                                out=dA,
                                in_=dt_o,
                                func=AF.Exp,
                                scale=a_neg[:, o, n : n + 1],
                            )
                            d1 = pc_d1.tile([128, TC], bf)
                            nc.vector.tensor_mul(
                                out=d1, in0=w_o, in1=B_bc[:, n, :]
                            )
                            h = pc_h.tile([128, TC], bf)
                            nc.vector.tensor_tensor_scan(
                                out=h,
                                data0=dA,
                                data1=d1,
                                initial=(
                                    0.0 if th == 0 else hstate[:, o, n : n + 1]
                                ),
                                op0=AO.mult,
                                op1=AO.add,
                            )
                            if th == 0:
                                nc.vector.tensor_copy(
                                    out=hstate[:, o, n : n + 1],
                                    in_=h[:, TC - 1 : TC],
                                )
                            tmp = pc_tmp.tile([128, TC], bf)
                            nc.vector.tensor_mul(out=tmp, in0=h, in1=C_bc[:, n, :])
                            for nn2 in range(2):
                                nc.tensor.matmul(
                                    y_ps[:, 512 * nn2 : 512 * (nn2 + 1)],
                                    lhsT=ident16,
                                    rhs=tmp[:, 512 * nn2 : 512 * (nn2 + 1)],
                                    start=(n == 0),
                                    stop=(n == D_STATE - 1),
                                )
                        g1 = pc_g.tile([128, TC], bf)
                        nc.vector.scalar_tensor_tensor(
                            out=g1,
                            in0=xs_o,
                            scalar=dvec[:, o : o + 1],
                            in1=y_ps,
                            op0=AO.mult,
                            op1=AO.add,
                        )
                        nc.vector.tensor_mul(
                            out=yg_all[:, o, :], in0=g1, in1=zs_o
                        )

                    # ---- Phase D (out_proj) for this t-half ----
                    for m in range(NMT):
                        op_a = ps_o.tile([128, 512], f32, tag="op_a")
                        op_b = ps_o.tile([128, 512], f32, tag="op_b")
                        ops = [op_a, op_b]
                        for o in range(NDT):
                            owt = pd_w.tile([128, 128], bf)
                            nc.sync.dma_start(
                                out=owt,
                                in_=ow16[o, :, 128 * m : 128 * (m + 1)],
                            )
                            for nn2 in range(2):
                                nc.tensor.matmul(
                                    ops[nn2],
                                    lhsT=owt,
                                    rhs=yg_all[:, o, 512 * nn2 : 512 * (nn2 + 1)],
                                    start=(o == 0),
                                    stop=(o == NDT - 1),
                                )
                        for nn2 in range(2):
                            ot_sb = pd_w.tile([128, 512], f32, tag="ot_sb")
                            nc.scalar.copy(out=ot_sb, in_=ops[nn2])
                            nc.sync.dma_start(
                                out=yT[
                                    128 * m : 128 * (m + 1),
                                    TC * th + 512 * nn2 : TC * th + 512 * (nn2 + 1),
                                ],
                                in_=ot_sb,
                            )

    return nc


def _get_nc():
    if "nc" not in _CACHE:
        _CACHE["nc"] = _build()
    return _CACHE["nc"]


def kernel(x, ln1_g, ln1_b, ln2_g, ln2_b, in_proj_w, conv_w, conv_b,
           x_proj_w, dt_proj_w, dt_proj_b, A_log, D, out_proj_w):
    from concourse.bass_utils import run_bass_kernel_spmd

    x = np.asarray(x, np.float32)
    f = np.float32
    shared = {
        "ipwT": np.ascontiguousarray(np.asarray(in_proj_w, f).T),
        "xpwT": np.ascontiguousarray(np.asarray(x_proj_w, f).T),
        "dtwT": np.ascontiguousarray(np.asarray(dt_proj_w, f).T),
        "dtb": np.ascontiguousarray(np.asarray(dt_proj_b, f)),
        "convw": np.ascontiguousarray(np.asarray(conv_w, f)),
        "convb": np.ascontiguousarray(np.asarray(conv_b, f)),
        "alog": np.ascontiguousarray(np.asarray(A_log, f)),
        "dvec": np.ascontiguousarray(np.asarray(D, f)),
        "owT": np.ascontiguousarray(np.asarray(out_proj_w, f).T),
    }
    g1, b1 = np.asarray(ln1_g, f), np.asarray(ln1_b, f)
    g2, b2 = np.asarray(ln2_g, f), np.asarray(ln2_b, f)

    in_maps = []
    for c in range(NCORES):
        b = c % NB
        if c < NB:
            xi = np.ascontiguousarray(x[b])
            lg, lb = g1, b1
        else:
            xi = np.ascontiguousarray(x[b][::-1])
            lg, lb = g2, b2
        m = dict(shared)
        m["xin"] = xi
        m["ln_g"] = np.ascontiguousarray(lg)
        m["ln_b"] = np.ascontiguousarray(lb)
        in_maps.append(m)

    nc = _get_nc()
    res = run_bass_kernel_spmd(nc, in_maps, core_ids=list(range(NCORES)))
    _CACHE["last_results"] = res

    out = x.copy()
    for c in range(NCORES):
        b = c % NB
        yt = res.results[c]["yT"]  # [D_MODEL, L]
        if c < NB:
            out[b] += yt.T
        else:
            out[b] += yt.T[::-1]
    return out


if __name__ == "__main__":
    ins = dict(np.load("/root/problem/inputs.npz"))
    o = kernel(**ins)
    print("kernel ran, out shape", o.shape)
